# revision 22
# baseline (speedup 1.0000x reference)
"""CrossTuckerLayer kernel for 8x Trainium2 NeuronCores (Bass/Tile).

Computes y = einsum('bnvade,ABCDEF,oA,pB,qC,aD,dE,eF->bnvopq', ...)
reshaped to [b, n, v, o*p, q], data-parallel over the 2048 (b,n,v) samples
(256 per core). The kernel is DMA-bound (8 MiB x in + 16 MiB y out per
core, all bf16), so everything is folded host-side into two matrices:

  M1  [16384, 8] = (a0 (x) a1 (x) a2) . core^T  -- the full input-side
      reduction; stored as [k=128, (c, z)] so that for each inner-chunk
      column c the slice M1[:, c*8:(c+1)*8] is a [128, 8] stationary
      operand.
  Wout [8, 32768] = u0 (x) u1 (x) u2            -- the rank-8 expansion.

Per 128-sample window:
  reduce:  s2[z, s] = sum_c sum_k M1[k, c, z] * x[k, (c, s)] -- 128
      accumulating PE matmuls (tiny [128, 8] stationary, [128, 128] bf16
      moving) into one PSUM tile. No intermediate stages.
  expand:  y[s, :] = s2^T . Wout -- 64 PE matmuls ([8, 128] stationary,
      [8, 512] bf16 moving), PSUM -> bf16 SBUF (wide 1024-col copies
      alternating vector/scalar), 2 MiB contiguous SWDGE stores.

x is pre-transposed/pre-cast host-side into contiguous 1 MiB DMA chunks,
all pre-issued on the sync HWDGE ring (expand matmuls are emitted ahead
of the next window's reduce so the in-order PE queue never head-of-line
blocks on an x DMA); y is written in contiguous 2 MiB tiles on the
gpsimd/SWDGE ring and re-laid-out on host. Wout loads as two parallel
halves (scalar + gpsimd rings) since its 8-partition transfer is slow.
bf16 end-to-end keeps rel-err ~3e-3, well under the 2e-2 gate.
"""

import numpy as np
import ml_dtypes

import concourse.bass as bass
import concourse.bacc as bacc
import concourse.mybir as mybir
from concourse.tile import TileContext
from concourse.bass_utils import run_bass_kernel_spmd

F32 = mybir.dt.float32
BF16 = mybir.dt.bfloat16
BF = ml_dtypes.bfloat16

NCORES = 8
S_TOT = 2048          # 4*64*8 samples
S = S_TOT // NCORES   # 256 per core
FIN = 16 * 16 * 64    # 16384
FOUT = 256 * 128      # 32768
WIN = 128             # samples per window (stage-C out partition dim)
N_WIN = S // WIN      # 2
FBLK = 32             # f-columns per x DMA chunk ([128, FBLK*WIN] bf16 = 1MB)
N_FBLK = 128 // FBLK  # 4 chunks per window
YCHUNK = 512          # psum bank cols (fp32)
YSTAGE = 4096         # cols per expand group
YPAIR = 2 * YSTAGE    # cols per y staging tile / 2MB output DMA
N_YPAIR = FOUT // YPAIR           # 4 per window
NC_PER_YSTAGE = YSTAGE // YCHUNK  # 8


def _host_weights(core, u0, u1, u2, a0, a1, a2):
    """Fold the Tucker factors into M1 [128, 128*8] and Wout [8, FOUT]."""
    a0 = a0.astype(np.float64)
    a1 = a1.astype(np.float64)
    a2 = a2.astype(np.float64)
    # K3 [(a,d,e)=16384, (D,E,F)=8]
    K3 = np.einsum("aD,dE,eF->adeDEF", a0, a1, a2).reshape(FIN, 8)
    core_mat = core.astype(np.float64).reshape(8, 8)  # [(A,B,C), (D,E,F)]
    M1 = K3 @ core_mat.T  # [16384, (A,B,C)=8]
    M1sb = M1.reshape(128, 128 * 8)  # [k, (c, z)]
    Wout = np.einsum(
        "oA,pB,qC->ABCopq",
        u0.astype(np.float64), u1.astype(np.float64), u2.astype(np.float64),
    ).reshape(8, FOUT)
    return M1sb.astype(BF), np.ascontiguousarray(Wout.astype(BF))


def _host_x(x_flat_bf):
    """Per-core x chunks: [8*128, FBLK*WIN] bf16, chunk id = (w, fb)."""
    out = []
    for i in range(NCORES):
        xc = x_flat_bf[i * S:(i + 1) * S]          # [256, 16384]
        xt = xc.reshape(N_WIN, WIN, 128, N_FBLK, FBLK)  # (w, sw, k, fb, j)
        xt = xt.transpose(0, 3, 2, 4, 1)           # (w, fb, k, j, sw)
        out.append(np.ascontiguousarray(
            xt.reshape(N_WIN * N_FBLK * 128, FBLK * WIN)))
    return out


def _build():
    nc = bacc.Bacc("TRN2", target_bir_lowering=False, debug=False)
    x_d = nc.dram_tensor(
        "x", [N_WIN * N_FBLK * 128, FBLK * WIN], BF16, kind="ExternalInput")
    m1_d = nc.dram_tensor("m1", [128, 128 * 8], BF16, kind="ExternalInput")
    wo_d = nc.dram_tensor("wo", [8, FOUT], BF16, kind="ExternalInput")
    y_d = nc.dram_tensor(
        "y", [N_WIN * N_YPAIR * WIN, YPAIR], BF16, kind="ExternalOutput")

    with TileContext(nc) as tc:
        with (
            tc.tile_pool(name="consts", bufs=1) as cpool,
            tc.tile_pool(name="xp", bufs=8) as xp,
            tc.tile_pool(name="s2p", bufs=2) as s2p,
            tc.tile_pool(name="yp", bufs=3) as yp,
            tc.tile_pool(name="psB", bufs=2, space=bass.MemorySpace.PSUM) as psB,
            tc.tile_pool(name="psC", bufs=3, space=bass.MemorySpace.PSUM) as psC,
        ):
            # All x DMAs pre-issued up front on the sync ring. wo loads
            # as two parallel halves (scalar + SWDGE rings): its
            # 8-partition transfer only sustains ~52GB/s, so halving the
            # span matters.
            m1 = cpool.tile([128, 128 * 8], BF16)
            nc.sync.dma_start(m1[:], m1_d[:])
            wo = cpool.tile([8, FOUT], BF16)
            nc.gpsimd.dma_start(wo[:, FOUT // 2:], wo_d[:, FOUT // 2:])
            x_ts = [xp.tile([128, FBLK * WIN], BF16, tag="x", name="x_t")
                    for _ in range(N_WIN * N_FBLK)]

            def x_load(ci, eng):
                eng.dma_start(x_ts[ci][:], x_d[ci * 128:(ci + 1) * 128, :])

            nc.scalar.dma_start(wo[:, :FOUT // 2], wo_d[:, :FOUT // 2])
            for ci in range(N_WIN * N_FBLK):
                x_load(ci, nc.sync)

            def emit_reduce_chunk(w, fb, s2_ps):
                x_t = x_ts[w * N_FBLK + fb]
                for j in range(FBLK):
                    f = fb * FBLK + j
                    nc.tensor.matmul(
                        s2_ps[:],
                        m1[:, f * 8:(f + 1) * 8],
                        x_t[:, j * WIN:(j + 1) * WIN],
                        start=(f == 0), stop=(f == 127),
                    )

            def emit_s2(s2_ps):
                s2 = s2p.tile([8, WIN], BF16, tag="s2", name="s2")
                nc.vector.tensor_copy(s2[:], s2_ps[:])
                return s2

            copy_rot = [0]

            def emit_ypair(w, p2, s2):
                # two expand groups -> one [128, 8192] staging tile -> one
                # contiguous 2MB store on the SWDGE (gpsimd) ring
                y_sb = yp.tile([128, YPAIR], BF16, tag="ysb", name="y_sb")
                for sub in range(2):
                    st = p2 * 2 + sub
                    for cw in range(NC_PER_YSTAGE // 2):
                        # two matmuls fill a 2-bank psum tile, one wide copy
                        # (alternating vector/scalar) drains it
                        y_ps = psC.tile([128, 2 * YCHUNK], F32, tag="yps",
                                        name="y_ps")
                        for h in range(2):
                            c = st * NC_PER_YSTAGE + cw * 2 + h
                            nc.tensor.matmul(
                                y_ps[:, h * YCHUNK:(h + 1) * YCHUNK], s2[:],
                                wo[:, c * YCHUNK:(c + 1) * YCHUNK],
                                start=True, stop=True,
                            )
                        dst = y_sb[:, (sub * 4 + cw) * 2 * YCHUNK:
                                   (sub * 4 + cw + 1) * 2 * YCHUNK]
                        if copy_rot[0] % 2 == 0:
                            nc.vector.tensor_copy(dst, y_ps[:])
                        else:
                            nc.scalar.copy(dst, y_ps[:])
                        copy_rot[0] += 1
                ti = w * N_YPAIR + p2
                nc.gpsimd.dma_start(
                    y_d[ti * WIN:(ti + 1) * WIN, :], y_sb[:])

            # both reduces run back-to-back first (dense warm PE stream,
            # fully overlapped with the pre-issued x DMAs), then the
            # expands form one continuous 8-pair stream with no reduce
            # matmuls head-of-line blocking the in-order PE queue
            s2_ps0 = psB.tile([8, WIN], F32, tag="s2ps", name="s2_ps0")
            for fb in range(N_FBLK):
                emit_reduce_chunk(0, fb, s2_ps0)
            s20 = emit_s2(s2_ps0)
            s2_ps1 = psB.tile([8, WIN], F32, tag="s2ps", name="s2_ps1")
            for fb in range(N_FBLK):
                emit_reduce_chunk(1, fb, s2_ps1)
            s21 = emit_s2(s2_ps1)
            for p2 in range(N_YPAIR):
                emit_ypair(0, p2, s20)
            for p2 in range(N_YPAIR):
                emit_ypair(1, p2, s21)
    nc.compile()
    return nc


_NC_CACHE = []


def _get_nc():
    if not _NC_CACHE:
        _NC_CACHE.append(_build())
    return _NC_CACHE[0]


def run(inputs, trace=False):
    x = np.ascontiguousarray(np.asarray(inputs["x"], dtype=np.float32))
    M1sb, Wout = _host_weights(
        np.asarray(inputs["core"]),
        np.asarray(inputs["u0"]), np.asarray(inputs["u1"]),
        np.asarray(inputs["u2"]),
        np.asarray(inputs["a0"]), np.asarray(inputs["a1"]),
        np.asarray(inputs["a2"]),
    )
    x_flat_bf = x.reshape(S_TOT, FIN).astype(BF)
    x_cores = _host_x(x_flat_bf)
    nc = _get_nc()
    in_maps = []
    for i in range(NCORES):
        in_maps.append({"x": x_cores[i], "m1": M1sb, "wo": Wout})
    res = run_bass_kernel_spmd(
        nc, in_maps, core_ids=list(range(NCORES)), trace=trace,
    )
    ys = []
    for r in res.results:
        yc = np.asarray(r["y"]).reshape(N_WIN, N_YPAIR, WIN, YPAIR)
        yc = yc.transpose(0, 2, 1, 3).reshape(S, FOUT)
        ys.append(yc)
    y = np.concatenate(ys, axis=0).astype(np.float32)
    return y.reshape(4, 64, 8, 256, 128), res


def kernel(**inputs) -> np.ndarray:
    y, _ = run(inputs, trace=False)
    return y


# revision 24
# speedup vs baseline: 1.0700x; 1.0700x over previous
"""CrossTuckerLayer kernel for 8x Trainium2 NeuronCores (Bass/Tile).

Computes y = einsum('bnvade,ABCDEF,oA,pB,qC,aD,dE,eF->bnvopq', ...)
reshaped to [b, n, v, o*p, q], data-parallel over the 2048 (b,n,v) samples
(256 per core). The kernel is DMA-bound (8 MiB x in + 16 MiB y out per
core, all bf16), so everything is folded host-side into two matrices:

  M1  [16384, 8] = (a0 (x) a1 (x) a2) . core^T  -- the full input-side
      reduction; stored as [k=128, (c, z)] so that for each inner-chunk
      column c the slice M1[:, c*8:(c+1)*8] is a [128, 8] stationary
      operand.
  Wout [8, 32768] = u0 (x) u1 (x) u2            -- the rank-8 expansion.

Per 128-sample window:
  reduce:  s2[z, s] = sum_c sum_k M1[k, c, z] * x[k, (c, s)] -- 128
      accumulating PE matmuls (tiny [128, 8] stationary, [128, 128] bf16
      moving) into one PSUM tile. No intermediate stages.
  expand:  y[s, :] = s2^T . Wout -- 64 PE matmuls ([8, 128] stationary,
      [8, 512] bf16 moving), PSUM -> bf16 SBUF (wide 1024-col copies
      alternating vector/scalar), 2 MiB contiguous SWDGE stores.

x is pre-transposed/pre-cast host-side into contiguous 1 MiB DMA chunks,
all pre-issued on the sync HWDGE ring (expand matmuls are emitted ahead
of the next window's reduce so the in-order PE queue never head-of-line
blocks on an x DMA); y is written in contiguous 2 MiB tiles on the
gpsimd/SWDGE ring and re-laid-out on host. Wout loads as two parallel
halves (scalar + gpsimd rings) since its 8-partition transfer is slow.
bf16 end-to-end keeps rel-err ~3e-3, well under the 2e-2 gate.
"""

import numpy as np
import ml_dtypes

import concourse.bass as bass
import concourse.bacc as bacc
import concourse.mybir as mybir
from concourse.tile import TileContext
from concourse.bass_utils import run_bass_kernel_spmd

F32 = mybir.dt.float32
BF16 = mybir.dt.bfloat16
BF = ml_dtypes.bfloat16

NCORES = 8
S_TOT = 2048          # 4*64*8 samples
S = S_TOT // NCORES   # 256 per core
FIN = 16 * 16 * 64    # 16384
FOUT = 256 * 128      # 32768
WIN = 128             # samples per window (stage-C out partition dim)
N_WIN = S // WIN      # 2
FBLK = 32             # f-columns per x DMA chunk ([128, FBLK*WIN] bf16 = 1MB)
N_FBLK = 128 // FBLK  # 4 chunks per window
YCHUNK = 512          # psum bank cols (fp32)
YSTAGE = 4096         # cols per expand group
YPAIR = 2 * YSTAGE    # cols per y staging tile / 2MB output DMA
N_YPAIR = FOUT // YPAIR           # 4 per window
NC_PER_YSTAGE = YSTAGE // YCHUNK  # 8


def _host_weights(core, u0, u1, u2, a0, a1, a2):
    """Fold the Tucker factors into M1 [128, 128*8] and Wout [8, FOUT]."""
    a0 = a0.astype(np.float64)
    a1 = a1.astype(np.float64)
    a2 = a2.astype(np.float64)
    # K3 [(a,d,e)=16384, (D,E,F)=8]
    K3 = np.einsum("aD,dE,eF->adeDEF", a0, a1, a2).reshape(FIN, 8)
    core_mat = core.astype(np.float64).reshape(8, 8)  # [(A,B,C), (D,E,F)]
    M1 = K3 @ core_mat.T  # [16384, (A,B,C)=8]
    M1sb = M1.reshape(128, 128 * 8)  # [k, (c, z)]
    Wout = np.einsum(
        "oA,pB,qC->ABCopq",
        u0.astype(np.float64), u1.astype(np.float64), u2.astype(np.float64),
    ).reshape(8, FOUT)
    return M1sb.astype(BF), np.ascontiguousarray(Wout.astype(BF))


def _host_x(x_flat_bf):
    """Per-core x chunks: [8*128, FBLK*WIN] bf16, chunk id = (w, fb)."""
    out = []
    for i in range(NCORES):
        xc = x_flat_bf[i * S:(i + 1) * S]          # [256, 16384]
        xt = xc.reshape(N_WIN, WIN, 128, N_FBLK, FBLK)  # (w, sw, k, fb, j)
        xt = xt.transpose(0, 3, 2, 4, 1)           # (w, fb, k, j, sw)
        out.append(np.ascontiguousarray(
            xt.reshape(N_WIN * N_FBLK * 128, FBLK * WIN)))
    return out


def _build():
    nc = bacc.Bacc("TRN2", target_bir_lowering=False, debug=False)
    x_d = nc.dram_tensor(
        "x", [N_WIN * N_FBLK * 128, FBLK * WIN], BF16, kind="ExternalInput")
    m1_d = nc.dram_tensor("m1", [128, 128 * 8], BF16, kind="ExternalInput")
    wo_d = nc.dram_tensor("wo", [8, FOUT], BF16, kind="ExternalInput")
    y_d = nc.dram_tensor(
        "y", [N_WIN * N_YPAIR * WIN, YPAIR], BF16, kind="ExternalOutput")

    with TileContext(nc) as tc:
        with (
            tc.tile_pool(name="consts", bufs=1) as cpool,
            tc.tile_pool(name="xp", bufs=8) as xp,
            tc.tile_pool(name="s2p", bufs=2) as s2p,
            tc.tile_pool(name="yp", bufs=4) as yp,
            tc.tile_pool(name="psB", bufs=2, space=bass.MemorySpace.PSUM) as psB,
            tc.tile_pool(name="psC", bufs=3, space=bass.MemorySpace.PSUM) as psC,
        ):
            # All x DMAs pre-issued up front on the sync ring. wo loads
            # as two parallel halves (scalar + SWDGE rings): its
            # 8-partition transfer only sustains ~52GB/s, so halving the
            # span matters.
            m1 = cpool.tile([128, 128 * 8], BF16)
            nc.sync.dma_start(m1[:], m1_d[:])
            wo = cpool.tile([8, FOUT], BF16)
            nc.gpsimd.dma_start(wo[:, FOUT // 2:], wo_d[:, FOUT // 2:])
            x_ts = [xp.tile([128, FBLK * WIN], BF16, tag="x", name="x_t")
                    for _ in range(N_WIN * N_FBLK)]

            def x_load(ci, eng):
                eng.dma_start(x_ts[ci][:], x_d[ci * 128:(ci + 1) * 128, :])

            nc.scalar.dma_start(wo[:, :FOUT // 2], wo_d[:, :FOUT // 2])
            for ci in range(N_WIN * N_FBLK):
                x_load(ci, nc.sync)

            def emit_reduce_chunk(w, fb, s2_ps):
                x_t = x_ts[w * N_FBLK + fb]
                for j in range(FBLK):
                    f = fb * FBLK + j
                    nc.tensor.matmul(
                        s2_ps[:],
                        m1[:, f * 8:(f + 1) * 8],
                        x_t[:, j * WIN:(j + 1) * WIN],
                        start=(f == 0), stop=(f == 127),
                    )

            def emit_s2(s2_ps):
                s2 = s2p.tile([8, WIN], BF16, tag="s2", name="s2")
                nc.vector.tensor_copy(s2[:], s2_ps[:])
                return s2

            copy_rot = [0]

            def emit_ypair(w, p2, s2):
                # two expand groups -> one [128, 8192] staging tile -> one
                # contiguous 2MB store on the SWDGE (gpsimd) ring
                y_sb = yp.tile([128, YPAIR], BF16, tag="ysb", name="y_sb")
                for sub in range(2):
                    st = p2 * 2 + sub
                    for cw in range(NC_PER_YSTAGE // 2):
                        # two matmuls fill a 2-bank psum tile, one wide copy
                        # (alternating vector/scalar) drains it
                        y_ps = psC.tile([128, 2 * YCHUNK], F32, tag="yps",
                                        name="y_ps")
                        for h in range(2):
                            c = st * NC_PER_YSTAGE + cw * 2 + h
                            nc.tensor.matmul(
                                y_ps[:, h * YCHUNK:(h + 1) * YCHUNK], s2[:],
                                wo[:, c * YCHUNK:(c + 1) * YCHUNK],
                                start=True, stop=True,
                            )
                        dst = y_sb[:, (sub * 4 + cw) * 2 * YCHUNK:
                                   (sub * 4 + cw + 1) * 2 * YCHUNK]
                        if copy_rot[0] % 2 == 0:
                            nc.vector.tensor_copy(dst, y_ps[:])
                        else:
                            nc.scalar.copy(dst, y_ps[:])
                        copy_rot[0] += 1
                ti = w * N_YPAIR + p2
                nc.gpsimd.dma_start(
                    y_d[ti * WIN:(ti + 1) * WIN, :], y_sb[:])

            # window 0 reduce (x DMAs already in flight)
            s2_ps0 = psB.tile([8, WIN], F32, tag="s2ps", name="s2_ps0")
            for fb in range(N_FBLK):
                emit_reduce_chunk(0, fb, s2_ps0)
            s20 = emit_s2(s2_ps0)
            # window 0 expand interleaved with window 1 reduce; expand is
            # emitted first so the in-order PE queue never head-of-line
            # blocks ready expand matmuls behind reduce matmuls
            s2_ps1 = psB.tile([8, WIN], F32, tag="s2ps", name="s2_ps1")
            for p2 in range(N_YPAIR):
                emit_ypair(0, p2, s20)
                if p2 < N_FBLK:
                    emit_reduce_chunk(1, p2, s2_ps1)
            s21 = emit_s2(s2_ps1)
            for p2 in range(N_YPAIR):
                emit_ypair(1, p2, s21)
    nc.compile()
    return nc


_NC_CACHE = []


def _get_nc():
    if not _NC_CACHE:
        _NC_CACHE.append(_build())
    return _NC_CACHE[0]


def run(inputs, trace=False):
    x = np.ascontiguousarray(np.asarray(inputs["x"], dtype=np.float32))
    M1sb, Wout = _host_weights(
        np.asarray(inputs["core"]),
        np.asarray(inputs["u0"]), np.asarray(inputs["u1"]),
        np.asarray(inputs["u2"]),
        np.asarray(inputs["a0"]), np.asarray(inputs["a1"]),
        np.asarray(inputs["a2"]),
    )
    x_flat_bf = x.reshape(S_TOT, FIN).astype(BF)
    x_cores = _host_x(x_flat_bf)
    nc = _get_nc()
    in_maps = []
    for i in range(NCORES):
        in_maps.append({"x": x_cores[i], "m1": M1sb, "wo": Wout})
    res = run_bass_kernel_spmd(
        nc, in_maps, core_ids=list(range(NCORES)), trace=trace,
    )
    ys = []
    for r in res.results:
        yc = np.asarray(r["y"]).reshape(N_WIN, N_YPAIR, WIN, YPAIR)
        yc = yc.transpose(0, 2, 1, 3).reshape(S, FOUT)
        ys.append(yc)
    y = np.concatenate(ys, axis=0).astype(np.float32)
    return y.reshape(4, 64, 8, 256, 128), res


def kernel(**inputs) -> np.ndarray:
    y, _ = run(inputs, trace=False)
    return y
